# revision 30
# baseline (speedup 1.0000x reference)
"""AttentionBlock (GroupNorm + 1x1-conv QKV + spatial attention + 1x1-conv out
+ skip) on 8 Trainium2 NeuronCores.

Sharding: data-parallel over batch. B=16 -> 2 batches per core, weights
replicated, no collectives. Each core runs the same NEFF on its own batch
slice; the host gathers by concatenation.

v4 design (vs v3 baseline at ~115us):
  * Host folds the 1x1 convs:  M1 = W_q^T W_k  and  M2 = W_o W_v, so
        scores = xn^T M1 xn          (one projection t = M1 xn instead of q,k)
        out    = (M2 xn) attn^T + (W_o b_v + b_o)   (no separate v / proj_out)
    The bias fold is exact because softmax rows sum to 1.
  * Scores computed TRANSPOSED; softmax denominator Z via a ones-stationary
    matmul; normalization fused into output evacuation. (As v3.)
  * ACT-table discipline: the only ACT functions used are Exp/Square/Copy/
    Identity, all present in the `exp_and_others` set. A dummy Exp issued
    first forces that set resident once; Sqrt (not in the set) is never
    used - rstd comes from a DVE Newton iteration. v3 paid 3 table loads
    (2.7us each), one of them directly in the GroupNorm critical path.
  * GroupNorm chain batched: one [128,8] stats tile per batch ->
    ONE ind_dn matmul -> one short DVE chain on [8,8] -> ONE ind_up
    broadcast matmul -> 2-way split apply (Pool + DVE). v3 ran this
    per-chunk through 4 engines (~10us of pure latency per batch).
  * Z on all partitions directly: ones[128x128]-stationary matmul over
    E m-blocks accumulates Z[n] into every PSUM partition (v3's
    [1,n] Z + ACT copy + [1->128] broadcast matmul chain stalled PE).
  * Warm-up matmuls gated on arriving x chunks (fp32 junk matmuls on the
    x tiles): PE warmth tracks the DMA stream instead of burning all
    warm-up at t=8us and then idling >3.4us (which drops the HAM clock).
  * All five big matmul groups fp8e4 DoubleRow; exp biased into fp8 range
    (e^-SHIFT cancels in Z).

Layouts on chip (partition dim first):
  channels  c = 128*ct + p   (ct in 0..3)
  spatial   n = 128*mb + p   (mb in 0..7)
  x             [128, 4, 1024]  f32   ([c_part, ct, n])
  xn, t         [128, 4, 1024]  fp8   (t indexed [c_q, m])
  ut            [128, 8, 512]   fp8   ([m_part, mb, c_out])
  E = exp(S^T)  [128, 8, 1024]  fp8   ([m_part, mb, n])
  zr = 1/Z      [128, 2, 512]   f32   (Z broadcast over partitions)
"""

import os
import numpy as np

B, C, H, W = 16, 512, 32, 32
HW = H * W            # 1024
BL = 2                # batches per core
NCORES = 8
CT = C // 128         # 4 channel chunks
NBLK = HW // 128      # 8 spatial blocks
GSIZE = 16            # channels per group
GSLOT = 128 // GSIZE  # 8 groups per channel chunk
CNT = GSIZE * HW      # elements per group (16384)
EPS = 1e-5
INVSQ = float(1.0 / np.sqrt(np.float32(C)))
# exp(score/sqrt(C) - SHIFT): keeps exp output in fp8e4m3's range
# (max |score/sqrt(C)| ~ 6.2 -> exp <= ~110 < 240). e^-SHIFT cancels in Z.
SHIFT = float(os.environ.get("K_SHIFT", "1.5"))
# "fp8": DoubleRow fp8 for all big matmuls. "bf16": same structure, bf16.
V2DT = os.environ.get("K_V2DT", "fp8")
NWARM = int(os.environ.get("K_NWARM", "8"))
NEWTON = int(os.environ.get("K_NEWTON", "1"))

_CACHE = {}


def _build_program(need_bias):
    import concourse.bacc as bacc
    import concourse.tile as tile
    from concourse import mybir
    from concourse.tile_rust import add_dep_helper

    F32 = mybir.dt.float32
    Alu = mybir.AluOpType
    Act = mybir.ActivationFunctionType
    Ax = mybir.AxisListType
    BF16 = mybir.dt.bfloat16
    FP8 = V2DT == "fp8"
    CDT = mybir.dt.float8e4 if FP8 else BF16
    DR = mybir.MatmulPerfMode.DoubleRow if FP8 else None
    KSTEP = 2 if FP8 else 1   # kc contraction per matmul (DoubleRow pairs)
    NK = CT // KSTEP
    NM = NBLK // KSTEP

    nc = bacc.Bacc("TRN2", target_bir_lowering=False, debug=False)

    x_d = nc.dram_tensor("x", [BL, C, HW], BF16, kind="ExternalInput")
    m1_d = nc.dram_tensor("m1t", [C, C], CDT, kind="ExternalInput")
    m2_d = nc.dram_tensor("m2t", [C, C], CDT, kind="ExternalInput")
    gam_d = nc.dram_tensor("gamma_t", [128, CT], F32, kind="ExternalInput")
    bet_d = nc.dram_tensor("beta_t", [128, CT], F32, kind="ExternalInput")
    idn_dn_d = nc.dram_tensor("ind_dn", [128, GSLOT], F32, kind="ExternalInput")
    idn_up_d = nc.dram_tensor("ind_up", [GSLOT, 128], F32, kind="ExternalInput")
    if need_bias:
        c1_d = nc.dram_tensor("c1_t", [128, CT], F32, kind="ExternalInput")
        bf_d = nc.dram_tensor("bf_t", [128, CT], F32, kind="ExternalInput")
        wr_d = nc.dram_tensor("wr_t", [128, CT], CDT, kind="ExternalInput")
    out_d = nc.dram_tensor("out", [BL, C, HW], F32, kind="ExternalOutput")

    with tile.TileContext(nc) as tc:
        with (
            tc.tile_pool(name="consts", bufs=1) as cp,
            tc.tile_pool(name="work", bufs=1) as wp,
            tc.tile_pool(name="psum", bufs=1, space="PSUM") as pp,
        ):
            # ---- constants; memsets on DVE so Pool/ACT stay clear and the
            # first warm-up matmul is unblocked as soon as DVE's IRAM loads.
            warm = cp.tile([128, 512], BF16, name="warm", tag="warm")
            nc.vector.memset(warm[:], 1.0)
            ones_z = cp.tile([128, KSTEP, 128], CDT, name="ones_z", tag="ones_z")
            nc.vector.memset(ones_z[:], 1.0)
            ebias = cp.tile([128, 1], F32, name="ebias", tag="ebias")
            nc.vector.memset(ebias[:], -SHIFT)
            # dummy Exp: forces the exp_and_others ACT table set resident
            # before any real ACT op (everything else we use is a filler in
            # that set, so this is the only table load in the kernel).
            dexp = cp.tile([1, 1], F32, name="dexp", tag="dexp")
            nc.scalar.activation(dexp[:], ebias[0:1, 0:1], Act.Exp)

            def warmup(n, wdst=None):
                wps = pp.tile([128, 512], F32, name=f"wps{warmup.i}", tag="mm", bufs=6)
                warmup.i += 1
                for _ in range(n):
                    nc.tensor.matmul(wps[:], warm[:, 0:128], warm[:], start=True, stop=True)
            warmup.i = 0

            def warm_x(b, ct):
                # fp32 junk matmul on an arrived x chunk: keeps the PE HAM
                # window busy while tracking the DMA stream (no result use).
                wps = pp.tile([128, 512], F32, name=f"wx{b}_{ct}", tag="mm", bufs=6)
                xs = st[b]["x"]
                nc.tensor.matmul(wps[:], xs[:, ct, 0:128], xs[:, ct, 0:512], start=True, stop=True)

            # ---- small constants on the SWDGE queue ----
            ind_dn = cp.tile([128, GSLOT], F32, name="ind_dn", tag="ind_dn")
            nc.gpsimd.dma_start(ind_dn[:], idn_dn_d[:])
            ind_up = cp.tile([GSLOT, 128], F32, name="ind_up", tag="ind_up")
            nc.gpsimd.dma_start(ind_up[:], idn_up_d[:])
            gam = cp.tile([128, CT], F32, name="gam", tag="gam")
            nc.gpsimd.dma_start(gam[:], gam_d[:])
            bet = cp.tile([128, CT], F32, name="bet", tag="bet")
            nc.gpsimd.dma_start(bet[:], bet_d[:])
            if need_bias:
                c1 = cp.tile([128, CT], F32, name="c1", tag="c1")
                nc.gpsimd.dma_start(c1[:], c1_d[:])
                b_f = cp.tile([128, CT], F32, name="b_f", tag="b_f")
                nc.gpsimd.dma_start(b_f[:], bf_d[:])
                wr = cp.tile([128, CT], CDT, name="wr", tag="wr")
                nc.gpsimd.dma_start(wr[:], wr_d[:])

            st = [dict() for _ in range(BL)]

            def gn_stats(b, ct):
                # ssum[:, ct] = per-partition sum; ssum[:, 4+ct] = sum of sq.
                # b0 sum on DVE (idle early); b1 sum on ACT via Copy+accum
                # (ACT is otherwise idle in the x1-arrival window and Copy is
                # a filler in every table set). Sum-of-squares via ACT
                # Square+accumulator for both.
                s = st[b]
                if "ssum" not in s:
                    s["ssum"] = wp.tile([128, 2 * CT], F32, name=f"ssum{b}", tag="ssum", bufs=2)
                    s["scr"] = wp.tile([128, HW], CDT, name=f"scr{b}", tag="scr", bufs=2)
                    s["scr2"] = wp.tile([128, HW], CDT, name=f"scr2{b}", tag="scr2", bufs=2)
                if b == 0:
                    # spread b0 stats over all four engines so ssum closes
                    # right behind the x0 DMA: sums ct0/1 DVE, ct2/3 Pool
                    # (tensor_scalar+accum); squares ct0-2 ACT, ct3 fused
                    # square-reduce on DVE.
                    nc.vector.tensor_reduce(
                        out=s["ssum"][:, ct : ct + 1], in_=s["x"][:, ct, :],
                        axis=Ax.X, op=Alu.add,
                    )
                else:
                    # b1 sum on ACT (Copy+accum): keeps DVE free for the b0
                    # chain + evacuations; Copy is a filler in every table set.
                    # Pin behind b0's last Square so the scheduler cannot
                    # push b0's stats (the critical path) behind b1's.
                    cp_i = nc.scalar.activation(
                        s["scr"][:], s["x"][:, ct, :], Act.Copy,
                        accum_out=s["ssum"][:, ct : ct + 1],
                    )
                    if ct == 0 and "sq_last" in st[0]:
                        add_dep_helper(cp_i.ins, st[0]["sq_last"], reason="b0 stats first")
                sq_i = nc.scalar.activation(
                    s["scr"][:], s["x"][:, ct, :], Act.Square,
                    accum_out=s["ssum"][:, CT + ct : CT + ct + 1],
                )
                s["sq_last"] = sq_i.ins

            def gn_chain_a(b):
                # Batched for all 4 chunks: one tiny PE matmul folds the
                # group sums, one short DVE chain computes mean/rstd (Newton
                # rsqrt - no ACT Sqrt, no table switch).
                s = st[b]
                ps_g = pp.tile([GSLOT, 2 * CT], F32, name=f"psg{b}", tag="gbc", bufs=2)
                nc.tensor.matmul(ps_g[:], ind_dn[:], s["ssum"][:], start=True, stop=True)
                mr = s["mr"] = wp.tile([GSLOT, 2 * CT], F32, name=f"mr{b}", tag="mr", bufs=2)
                nc.vector.tensor_scalar_mul(mr[:], ps_g[:], 1.0 / CNT)  # [mean | E[x^2]]
                v2 = wp.tile([GSLOT, CT], F32, name=f"v2{b}", tag="v2", bufs=2)
                nc.vector.tensor_mul(v2[:], mr[:, 0:CT], mr[:, 0:CT])
                nc.vector.tensor_sub(v2[:], mr[:, CT : 2 * CT], v2[:])
                nc.vector.tensor_scalar_add(v2[:], v2[:], EPS)          # var + eps
                y = mr[:, CT : 2 * CT]
                nc.vector.reciprocal_approx_fast(y, v2[:])              # seed ~ 1/v
                u = wp.tile([GSLOT, CT], F32, name=f"u{b}", tag="u", bufs=2)
                nc.vector.tensor_scalar_mul(u[:], v2[:], 0.5)
                for it in range(NEWTON):                                # y -> 1/sqrt(v)
                    h = wp.tile([GSLOT, CT], F32, name=f"nh{b}_{it}", tag="nh", bufs=4)
                    nc.vector.tensor_mul(h[:], y, y)
                    nc.vector.tensor_mul(h[:], h[:], u[:])
                    nc.vector.tensor_scalar(
                        out=h[:], in0=h[:], scalar1=-1.0, scalar2=1.5,
                        op0=Alu.mult, op1=Alu.add,
                    )
                    nc.vector.tensor_mul(y, y, h[:])

            def gn_chain_b(b):
                # Broadcast mean/rstd back to 128 partitions (one PE matmul)
                # and form the per-channel a/b coefficients in 3 DVE ops.
                s = st[b]
                ps_bc = pp.tile([128, 2 * CT], F32, name=f"psbc{b}", tag="gbc", bufs=2)
                nc.tensor.matmul(ps_bc[:], ind_up[:], s["mr"][:], start=True, stop=True)
                ab = s["ab"] = wp.tile([128, 2 * CT], F32, name=f"ab{b}", tag="ab", bufs=2)
                nc.vector.tensor_mul(ab[:, 0:CT], ps_bc[:, CT : 2 * CT], gam[:])
                tb = wp.tile([128, CT], F32, name=f"tb{b}", tag="tb", bufs=2)
                nc.vector.tensor_mul(tb[:], ps_bc[:, 0:CT], ab[:, 0:CT])
                nc.vector.tensor_sub(ab[:, CT : 2 * CT], bet[:], tb[:])

            def gn_apply(b, ct):
                # xn = a*x + b in the compute dtype, spread over THREE
                # engines (Pool, DVE, ACT-Identity) so all four chunks
                # drain in ~2 tile-times.
                s = st[b]
                if "xn" not in s:
                    s["xn"] = wp.tile([128, CT, HW], CDT, name=f"xn{b}", tag="xn", bufs=2)
                ab = s["ab"]
                eng = nc.gpsimd if ct in (0, 3) else nc.vector
                eng.tensor_scalar(
                    out=s["xn"][:, ct, :], in0=s["x"][:, ct, :],
                    scalar1=ab[:, ct : ct + 1], scalar2=ab[:, CT + ct : CT + ct + 1],
                    op0=Alu.mult, op1=Alu.add,
                )

            def mm_k(ps, lhs_fn, rhs_fn, nk):
                for k in range(nk):
                    nc.tensor.matmul(
                        ps[:], lhs_fn(k), rhs_fn(k),
                        start=(k == 0), stop=(k == nk - 1),
                        perf_mode=DR,
                    )

            def ksl(t, k, lo, hi):
                return t[:, KSTEP * k : KSTEP * (k + 1), lo:hi]

            def t_mm(b, oc, nh):
                # t[:, oc, nh-half] = (M1 xn)[oc-chunk, half]  (+ c1 if biased)
                s = st[b]
                if "t" not in s:
                    s["t"] = wp.tile([128, CT, HW], CDT, name=f"t{b}", tag="t", bufs=2)
                ps = pp.tile([128, 512], F32, name=f"pt{b}_{oc}_{nh}", tag="mm", bufs=6)
                mm_k(ps,
                     lambda k: ksl(m1, k, oc * 128, (oc + 1) * 128),
                     lambda k: ksl(s["xn"], k, nh * 512, (nh + 1) * 512), NK)
                dst = s["t"][:, oc, nh * 512 : (nh + 1) * 512]
                if need_bias:
                    nc.scalar.activation(dst, ps[:], Act.Identity, bias=c1[:, oc : oc + 1])
                else:
                    nc.vector.tensor_copy(dst, ps[:])

            def ut_mm(b, mb):
                # ut[:, mb, :] = (xn^T M2^T)[mb-block, :]
                s = st[b]
                if "ut" not in s:
                    s["ut"] = wp.tile([128, NBLK, C], CDT, name=f"ut{b}", tag="ut", bufs=2)
                ps = pp.tile([128, 512], F32, name=f"pu{b}_{mb}", tag="mm", bufs=6)
                mm_k(ps,
                     lambda k: ksl(s["xn"], k, mb * 128, (mb + 1) * 128),
                     lambda k: ksl(m2, k, 0, C), NK)
                nc.scalar.copy(s["ut"][:, mb, :], ps[:])

            def rx_mm(b):
                # general-bias path: rx_t[p, mb] = sum_c wr[c] xn[c, m]; the
                # per-key exp bias is INVSQ*rx - SHIFT (+ bq.bk const).
                s = st[b]
                s["rxb"] = wp.tile([128, NBLK], F32, name=f"rxb{b}", tag="rxb", bufs=2)
                for mb in range(NBLK):
                    ps = pp.tile([128, 1], F32, name=f"prx{b}_{mb}", tag="gbc", bufs=2)
                    mm_k(ps,
                         lambda k: ksl(s["xn"], k, mb * 128, (mb + 1) * 128),
                         lambda k: ksl(wr, k, 0, 1), NK)
                    nc.vector.tensor_scalar(
                        out=s["rxb"][:, mb : mb + 1], in0=ps[:],
                        scalar1=INVSQ, scalar2=RXCONST[0] - SHIFT,
                        op0=Alu.mult, op1=Alu.add,
                    )

            def sc_mm(b, mb, nh):
                # scores^T tile [m-block, n-half] + exp -> E fp8
                s = st[b]
                if "E" not in s:
                    s["E"] = wp.tile([128, NBLK, HW], CDT, name=f"E{b}", tag="E", bufs=2)
                ps = pp.tile([128, 512], F32, name=f"psc{b}_{mb}_{nh}", tag="mm", bufs=6)
                mm_k(ps,
                     lambda k: ksl(s["t"], k, mb * 128, (mb + 1) * 128),
                     lambda k: ksl(s["xn"], k, nh * 512, (nh + 1) * 512), NK)
                bias = s["rxb"][:, mb : mb + 1] if need_bias else ebias[:, 0:1]
                nc.scalar.activation(
                    s["E"][:, mb, nh * 512 : (nh + 1) * 512], ps[:],
                    Act.Exp, bias=bias, scale=INVSQ,
                )

            def z_mm(b, nh):
                # Z[n] on ALL partitions at once: ones[128x128]-stationary
                # matmul accumulated over the 8 m-blocks; then 1/Z on DVE.
                s = st[b]
                if "zr" not in s:
                    s["zr"] = wp.tile([128, 2, 512], F32, name=f"zr{b}", tag="zr", bufs=2)
                ps = pp.tile([128, 512], F32, name=f"psz{b}_{nh}", tag="mm", bufs=6)
                mm_k(ps,
                     lambda k: ones_z[:] if FP8 else ones_z[:, 0, :],
                     lambda k: ksl(s["E"], k, nh * 512, (nh + 1) * 512), NM)
                nc.vector.reciprocal_approx_fast(s["zr"][:, nh, :], ps[:])

            def o_mm(b, ct, nh, fine=False):
                # out[ct-chunk, nh-half] = outU * zr (+ b_f) + skip, streamed
                # out. fine=True evacuates in 2x256 halves (shorter chain on
                # the kernel's very last tiles).
                s = st[b]
                out_r = out_d[b].rearrange("(ct p) n -> p ct n", p=128)
                ps = pp.tile([128, 512], F32, name=f"po{b}_{ct}_{nh}", tag="mm", bufs=6)
                mm_k(ps,
                     lambda k: ksl(s["ut"], k, ct * 128, (ct + 1) * 128),
                     lambda k: ksl(s["E"], k, nh * 512, (nh + 1) * 512), NM)
                tmp = wp.tile([128, 512], F32, name=f"tmp{b}_{ct}_{nh}", tag="tmp", bufs=4)
                nhalf = 2 if fine else 1
                for hh in range(nhalf):
                    w = 512 // nhalf
                    fsl = slice(hh * w, (hh + 1) * w)
                    sl = s["x"][:, ct, nh * 512 + hh * w : nh * 512 + (hh + 1) * w]
                    nc.vector.tensor_tensor(
                        tmp[:, fsl], ps[:, fsl], s["zr"][:, nh, fsl], op=Alu.mult)
                    if need_bias:
                        nc.vector.scalar_tensor_tensor(
                            out=tmp[:, fsl], in0=tmp[:, fsl],
                            scalar=b_f[:, ct : ct + 1], in1=sl,
                            op0=Alu.add, op1=Alu.add,
                        )
                    else:
                        # alternate Pool/DVE so consecutive skip-adds overlap
                        eng = nc.gpsimd if (ct + hh) % 2 == 0 else nc.vector
                        eng.tensor_tensor(tmp[:, fsl], tmp[:, fsl], sl, op=Alu.add)
                    nc.sync.dma_start(
                        out_r[:, ct, nh * 512 + hh * w : nh * 512 + (hh + 1) * w],
                        tmp[:, fsl])

            # ---- DMA emission order == HWDGE queue order: all of x0, then
            # m1 (needed ~when t(0) starts), then x1, then m2 (needed at
            # ut(0), which runs after sc(0,h0)).
            for b in range(BL):
                st[b]["x"] = wp.tile([128, CT, HW], BF16, name=f"x{b}", tag="x", bufs=2)
            x_r = [x_d[b].rearrange("(ct p) n -> p ct n", p=128) for b in range(BL)]
            for ct in range(CT):
                nc.sync.dma_start(st[0]["x"][:, ct, :], x_r[0][:, ct, :])
            m1 = cp.tile([128, CT, C], CDT, name="m1", tag="m1")
            nc.sync.dma_start(m1[:], m1_d.rearrange("(kc p) o -> p kc o", p=128))
            for ct in range(CT):
                nc.sync.dma_start(st[1]["x"][:, ct, :], x_r[1][:, ct, :])
            m2 = cp.tile([128, CT, C], CDT, name="m2", tag="m2")
            nc.sync.dma_start(m2[:], m2_d.rearrange("(kc p) o -> p kc o", p=128))

            # ---- emission order == scheduler priority ----
            # PE queue: warm-ups, gn(0) tiny mms, t(0), sc(0,h0), gn(1)
            # tiny mms (their stats are ready by then - no head-of-line
            # block), ut(0), sc(0,h1), t(1), then the attention tail.
            warmup(NWARM)
            for ct in range(CT):
                gn_stats(0, ct)
                warm_x(0, ct)
                warm_x(0, ct)
                warm_x(0, ct)
                warm_x(0, ct)
            gn_chain_a(0)
            warm_x(1, 0)
            warm_x(1, 1)
            gn_chain_b(0)
            for ct in range(CT):
                gn_apply(0, ct)
            for ct in range(CT):
                gn_stats(1, ct)
            warm_x(1, 2)
            warm_x(1, 3)
            if need_bias:
                rx_mm(0)
            for oc in range(CT):
                t_mm(0, oc, 0); t_mm(0, oc, 1)
            gn_chain_a(1)
            for mb in range(NBLK):
                sc_mm(0, mb, 0)
            gn_chain_b(1)
            for ct in range(CT):
                gn_apply(1, ct)
            for mb in range(NBLK):
                ut_mm(0, mb)
            for mb in range(NBLK):
                sc_mm(0, mb, 1)
            if need_bias:
                rx_mm(1)
            for oc in range(CT):
                t_mm(1, oc, 0); t_mm(1, oc, 1)
            z_mm(0, 0)
            for ct in range(CT):
                o_mm(0, ct, 0)
            z_mm(0, 1)
            for ct in range(CT):
                o_mm(0, ct, 1)
            for mb in range(NBLK):
                sc_mm(1, mb, 0)
            for mb in range(NBLK):
                ut_mm(1, mb)
            for mb in range(NBLK):
                sc_mm(1, mb, 1)
            z_mm(1, 0)
            for ct in range(CT):
                o_mm(1, ct, 0)
            z_mm(1, 1)
            for ct in range(CT):
                o_mm(1, ct, 1, fine=(ct >= 2))

    nc.compile()
    return nc


# constant exp-bias addend for the general-bias path (bq.bk term);
# set by _make_in_maps before the program is built
RXCONST = [0.0]


def _get_program(need_bias):
    key = (V2DT, SHIFT, need_bias, NEWTON, NWARM)
    if key not in _CACHE:
        _CACHE[key] = _build_program(need_bias)
    return _CACHE[key]


def _to_compute(a):
    """Convert host fp32 weights to the matmul compute format."""
    import ml_dtypes
    a = np.ascontiguousarray(a, dtype=np.float32)
    if V2DT == "fp8":
        return np.ascontiguousarray(a.astype(ml_dtypes.float8_e4m3))
    return np.ascontiguousarray(a.astype(ml_dtypes.bfloat16))


def _make_in_maps(x, gamma, beta, w_in, b_in, w_out, b_out):
    import ml_dtypes
    x = np.ascontiguousarray(
        np.asarray(x, dtype=np.float32).reshape(B, C, HW).astype(ml_dtypes.bfloat16)
    )
    w_in = np.asarray(w_in, dtype=np.float32)
    w_out = np.asarray(w_out, dtype=np.float32)
    b_in = np.asarray(b_in, dtype=np.float32)
    b_out = np.asarray(b_out, dtype=np.float32)
    wq, wk, wv = w_in[0:C], w_in[C : 2 * C], w_in[2 * C : 3 * C]
    bq, bk, bv = b_in[0:C], b_in[C : 2 * C], b_in[2 * C : 3 * C]
    m1 = wq.T @ wk                      # scores = xn^T m1 xn (+ bias terms)
    m2 = w_out @ wv                     # out = m2 xn attn^T + bf
    c1 = wq.T @ bk                      # q-side bias fold (per-channel)
    bf = w_out @ bv + b_out             # exact: softmax rows sum to 1
    wr = wk.T @ bq                      # k-side bias: varies along keys m
    need_bias = bool(np.any(c1) or np.any(bf) or np.any(wr) or np.any(bq))
    RXCONST[0] = float(INVSQ * np.dot(bq, bk))

    def cvec(v):
        return np.ascontiguousarray(v.reshape(CT, 128).T, dtype=np.float32)

    consts = {
        "m1t": _to_compute(m1.T),
        "m2t": _to_compute(m2.T),
        "gamma_t": cvec(np.asarray(gamma, dtype=np.float32)),
        "beta_t": cvec(np.asarray(beta, dtype=np.float32)),
        "ind_dn": (np.arange(128)[:, None] // GSIZE == np.arange(GSLOT)[None, :]).astype(np.float32),
        "ind_up": (np.arange(GSLOT)[:, None] == np.arange(128)[None, :] // GSIZE).astype(np.float32),
    }
    if need_bias:
        consts["c1_t"] = cvec(c1)
        consts["bf_t"] = cvec(bf)
        consts["wr_t"] = _to_compute(wr.reshape(CT, 128).T)
    return need_bias, [
        {"x": x[c * BL : (c + 1) * BL], **consts}
        for c in range(NCORES)
    ]


def run(inputs, trace=False):
    """Run on 8 cores; returns (output [B,C,H,W], BassKernelResults)."""
    from concourse.bass_utils import run_bass_kernel_spmd

    need_bias, in_maps = _make_in_maps(**inputs)
    nc = _get_program(need_bias)
    res = run_bass_kernel_spmd(nc, in_maps, core_ids=list(range(NCORES)), trace=trace)
    out = np.concatenate([res.results[i]["out"] for i in range(NCORES)], axis=0)
    return out.reshape(B, C, H, W).astype(np.float32), res


def kernel(**inputs) -> np.ndarray:
    out, _ = run(inputs)
    return out


# revision 31
# speedup vs baseline: 1.0437x; 1.0437x over previous
"""AttentionBlock (GroupNorm + 1x1-conv QKV + spatial attention + 1x1-conv out
+ skip) on 8 Trainium2 NeuronCores.

Sharding: data-parallel over batch. B=16 -> 2 batches per core, weights
replicated, no collectives. Each core runs the same NEFF on its own batch
slice; the host gathers by concatenation.

v4 design (vs v3 baseline at ~115us):
  * Host folds the 1x1 convs:  M1 = W_q^T W_k  and  M2 = W_o W_v, so
        scores = xn^T M1 xn          (one projection t = M1 xn instead of q,k)
        out    = (M2 xn) attn^T + (W_o b_v + b_o)   (no separate v / proj_out)
    The bias fold is exact because softmax rows sum to 1.
  * Scores computed TRANSPOSED; softmax denominator Z via a ones-stationary
    matmul; normalization fused into output evacuation. (As v3.)
  * ACT-table discipline: the only ACT functions used are Exp/Square/Copy/
    Identity, all present in the `exp_and_others` set. A dummy Exp issued
    first forces that set resident once; Sqrt (not in the set) is never
    used - rstd comes from a DVE Newton iteration. v3 paid 3 table loads
    (2.7us each), one of them directly in the GroupNorm critical path.
  * GroupNorm chain batched: one [128,8] stats tile per batch ->
    ONE ind_dn matmul -> one short DVE chain on [8,8] -> ONE ind_up
    broadcast matmul -> 2-way split apply (Pool + DVE). v3 ran this
    per-chunk through 4 engines (~10us of pure latency per batch).
  * Z on all partitions directly: ones[128x128]-stationary matmul over
    E m-blocks accumulates Z[n] into every PSUM partition (v3's
    [1,n] Z + ACT copy + [1->128] broadcast matmul chain stalled PE).
  * Warm-up matmuls gated on arriving x chunks (fp32 junk matmuls on the
    x tiles): PE warmth tracks the DMA stream instead of burning all
    warm-up at t=8us and then idling >3.4us (which drops the HAM clock).
  * All five big matmul groups fp8e4 DoubleRow; exp biased into fp8 range
    (e^-SHIFT cancels in Z).

Layouts on chip (partition dim first):
  channels  c = 128*ct + p   (ct in 0..3)
  spatial   n = 128*mb + p   (mb in 0..7)
  x             [128, 4, 1024]  f32   ([c_part, ct, n])
  xn, t         [128, 4, 1024]  fp8   (t indexed [c_q, m])
  ut            [128, 8, 512]   fp8   ([m_part, mb, c_out])
  E = exp(S^T)  [128, 8, 1024]  fp8   ([m_part, mb, n])
  zr = 1/Z      [128, 2, 512]   f32   (Z broadcast over partitions)
"""

import os
import numpy as np

B, C, H, W = 16, 512, 32, 32
HW = H * W            # 1024
BL = 2                # batches per core
NCORES = 8
CT = C // 128         # 4 channel chunks
NBLK = HW // 128      # 8 spatial blocks
GSIZE = 16            # channels per group
GSLOT = 128 // GSIZE  # 8 groups per channel chunk
CNT = GSIZE * HW      # elements per group (16384)
EPS = 1e-5
INVSQ = float(1.0 / np.sqrt(np.float32(C)))
# exp(score/sqrt(C) - SHIFT): keeps exp output in fp8e4m3's range
# (max |score/sqrt(C)| ~ 6.2 -> exp <= ~110 < 240). e^-SHIFT cancels in Z.
SHIFT = float(os.environ.get("K_SHIFT", "1.5"))
# "fp8": DoubleRow fp8 for all big matmuls. "bf16": same structure, bf16.
V2DT = os.environ.get("K_V2DT", "fp8")
NWARM = int(os.environ.get("K_NWARM", "8"))
NEWTON = int(os.environ.get("K_NEWTON", "1"))

_CACHE = {}


def _build_program(need_bias):
    import concourse.bacc as bacc
    import concourse.tile as tile
    from concourse import mybir
    from concourse.tile_rust import add_dep_helper

    F32 = mybir.dt.float32
    Alu = mybir.AluOpType
    Act = mybir.ActivationFunctionType
    Ax = mybir.AxisListType
    BF16 = mybir.dt.bfloat16
    FP8 = V2DT == "fp8"
    CDT = mybir.dt.float8e4 if FP8 else BF16
    DR = mybir.MatmulPerfMode.DoubleRow if FP8 else None
    KSTEP = 2 if FP8 else 1   # kc contraction per matmul (DoubleRow pairs)
    NK = CT // KSTEP
    NM = NBLK // KSTEP

    nc = bacc.Bacc("TRN2", target_bir_lowering=False, debug=False)

    x_d = nc.dram_tensor("x", [BL, C, HW], BF16, kind="ExternalInput")
    m1_d = nc.dram_tensor("m1t", [C, C], CDT, kind="ExternalInput")
    m2_d = nc.dram_tensor("m2t", [C, C], CDT, kind="ExternalInput")
    gam_d = nc.dram_tensor("gamma_t", [128, CT], F32, kind="ExternalInput")
    bet_d = nc.dram_tensor("beta_t", [128, CT], F32, kind="ExternalInput")
    idn_dn_d = nc.dram_tensor("ind_dn", [128, GSLOT], F32, kind="ExternalInput")
    idn_up_d = nc.dram_tensor("ind_up", [GSLOT, 128], F32, kind="ExternalInput")
    if need_bias:
        c1_d = nc.dram_tensor("c1_t", [128, CT], F32, kind="ExternalInput")
        bf_d = nc.dram_tensor("bf_t", [128, CT], F32, kind="ExternalInput")
        wr_d = nc.dram_tensor("wr_t", [128, CT], CDT, kind="ExternalInput")
    out_d = nc.dram_tensor("out", [BL, C, HW], F32, kind="ExternalOutput")

    with tile.TileContext(nc) as tc:
        with (
            tc.tile_pool(name="consts", bufs=1) as cp,
            tc.tile_pool(name="work", bufs=1) as wp,
            tc.tile_pool(name="psum", bufs=1, space="PSUM") as pp,
        ):
            # ---- constants; memsets on DVE so Pool/ACT stay clear and the
            # first warm-up matmul is unblocked as soon as DVE's IRAM loads.
            warm = cp.tile([128, 512], BF16, name="warm", tag="warm")
            nc.vector.memset(warm[:], 1.0)
            ones_z = cp.tile([128, KSTEP, 128], CDT, name="ones_z", tag="ones_z")
            nc.vector.memset(ones_z[:], 1.0)
            ebias = cp.tile([128, 1], F32, name="ebias", tag="ebias")
            nc.vector.memset(ebias[:], -SHIFT)
            # dummy Exp: forces the exp_and_others ACT table set resident
            # before any real ACT op (everything else we use is a filler in
            # that set, so this is the only table load in the kernel).
            dexp = cp.tile([1, 1], F32, name="dexp", tag="dexp")
            nc.scalar.activation(dexp[:], ebias[0:1, 0:1], Act.Exp)

            def warmup(n, wdst=None):
                wps = pp.tile([128, 512], F32, name=f"wps{warmup.i}", tag="mm", bufs=7)
                warmup.i += 1
                for _ in range(n):
                    nc.tensor.matmul(wps[:], warm[:, 0:128], warm[:], start=True, stop=True)
            warmup.i = 0

            def warm_x(b, ct):
                # fp32 junk matmul on an arrived x chunk: keeps the PE HAM
                # window busy while tracking the DMA stream (no result use).
                wps = pp.tile([128, 512], F32, name=f"wx{b}_{ct}", tag="mm", bufs=7)
                xs = st[b]["x"]
                nc.tensor.matmul(wps[:], xs[:, ct, 0:128], xs[:, ct, 0:512], start=True, stop=True)

            # ---- small constants on the SWDGE queue ----
            ind_dn = cp.tile([128, GSLOT], F32, name="ind_dn", tag="ind_dn")
            nc.gpsimd.dma_start(ind_dn[:], idn_dn_d[:])
            ind_up = cp.tile([GSLOT, 128], F32, name="ind_up", tag="ind_up")
            nc.gpsimd.dma_start(ind_up[:], idn_up_d[:])
            gam = cp.tile([128, CT], F32, name="gam", tag="gam")
            nc.gpsimd.dma_start(gam[:], gam_d[:])
            bet = cp.tile([128, CT], F32, name="bet", tag="bet")
            nc.gpsimd.dma_start(bet[:], bet_d[:])
            if need_bias:
                c1 = cp.tile([128, CT], F32, name="c1", tag="c1")
                nc.gpsimd.dma_start(c1[:], c1_d[:])
                b_f = cp.tile([128, CT], F32, name="b_f", tag="b_f")
                nc.gpsimd.dma_start(b_f[:], bf_d[:])
                wr = cp.tile([128, CT], CDT, name="wr", tag="wr")
                nc.gpsimd.dma_start(wr[:], wr_d[:])

            st = [dict() for _ in range(BL)]

            def gn_stats(b, ct):
                # ssum[:, ct] = per-partition sum; ssum[:, 4+ct] = sum of sq.
                # b0 sum on DVE (idle early); b1 sum on ACT via Copy+accum
                # (ACT is otherwise idle in the x1-arrival window and Copy is
                # a filler in every table set). Sum-of-squares via ACT
                # Square+accumulator for both.
                s = st[b]
                if "ssum" not in s:
                    s["ssum"] = wp.tile([128, 2 * CT], F32, name=f"ssum{b}", tag="ssum", bufs=2)
                    s["scr"] = wp.tile([128, HW], CDT, name=f"scr{b}", tag="scr", bufs=2)
                    s["scr2"] = wp.tile([128, HW], CDT, name=f"scr2{b}", tag="scr2", bufs=2)
                if b == 0:
                    # spread b0 stats over all four engines so ssum closes
                    # right behind the x0 DMA: sums ct0/1 DVE, ct2/3 Pool
                    # (tensor_scalar+accum); squares ct0-2 ACT, ct3 fused
                    # square-reduce on DVE.
                    nc.vector.tensor_reduce(
                        out=s["ssum"][:, ct : ct + 1], in_=s["x"][:, ct, :],
                        axis=Ax.X, op=Alu.add,
                    )
                else:
                    # b1 sum on ACT (Copy+accum): keeps DVE free for the b0
                    # chain + evacuations; Copy is a filler in every table set.
                    # Pin behind b0's last Square so the scheduler cannot
                    # push b0's stats (the critical path) behind b1's.
                    cp_i = nc.scalar.activation(
                        s["scr"][:], s["x"][:, ct, :], Act.Copy,
                        accum_out=s["ssum"][:, ct : ct + 1],
                    )
                    if ct == 0 and "sq_last" in st[0]:
                        add_dep_helper(cp_i.ins, st[0]["sq_last"], reason="b0 stats first")
                sq_i = nc.scalar.activation(
                    s["scr"][:], s["x"][:, ct, :], Act.Square,
                    accum_out=s["ssum"][:, CT + ct : CT + ct + 1],
                )
                s["sq_last"] = sq_i.ins

            def gn_chain_a(b):
                # Batched for all 4 chunks: one tiny PE matmul folds the
                # group sums, one short DVE chain computes mean/rstd (Newton
                # rsqrt - no ACT Sqrt, no table switch).
                s = st[b]
                ps_g = pp.tile([GSLOT, 2 * CT], F32, name=f"psg{b}", tag="gbc", bufs=1)
                nc.tensor.matmul(ps_g[:], ind_dn[:], s["ssum"][:], start=True, stop=True)
                mr = s["mr"] = wp.tile([GSLOT, 2 * CT], F32, name=f"mr{b}", tag="mr", bufs=2)
                nc.vector.tensor_scalar_mul(mr[:], ps_g[:], 1.0 / CNT)  # [mean | E[x^2]]
                v2 = wp.tile([GSLOT, CT], F32, name=f"v2{b}", tag="v2", bufs=2)
                nc.vector.tensor_mul(v2[:], mr[:, 0:CT], mr[:, 0:CT])
                nc.vector.tensor_sub(v2[:], mr[:, CT : 2 * CT], v2[:])
                nc.vector.tensor_scalar_add(v2[:], v2[:], EPS)          # var + eps
                y = mr[:, CT : 2 * CT]
                nc.vector.reciprocal_approx_fast(y, v2[:])              # seed ~ 1/v
                u = wp.tile([GSLOT, CT], F32, name=f"u{b}", tag="u", bufs=2)
                nc.vector.tensor_scalar_mul(u[:], v2[:], 0.5)
                for it in range(NEWTON):                                # y -> 1/sqrt(v)
                    h = wp.tile([GSLOT, CT], F32, name=f"nh{b}_{it}", tag="nh", bufs=4)
                    nc.vector.tensor_mul(h[:], y, y)
                    nc.vector.tensor_mul(h[:], h[:], u[:])
                    nc.vector.tensor_scalar(
                        out=h[:], in0=h[:], scalar1=-1.0, scalar2=1.5,
                        op0=Alu.mult, op1=Alu.add,
                    )
                    nc.vector.tensor_mul(y, y, h[:])

            def gn_chain_b(b):
                # Broadcast mean/rstd back to 128 partitions (one PE matmul)
                # and form the per-channel a/b coefficients in 3 DVE ops.
                s = st[b]
                ps_bc = pp.tile([128, 2 * CT], F32, name=f"psbc{b}", tag="gbc", bufs=1)
                nc.tensor.matmul(ps_bc[:], ind_up[:], s["mr"][:], start=True, stop=True)
                ab = s["ab"] = wp.tile([128, 2 * CT], F32, name=f"ab{b}", tag="ab", bufs=2)
                nc.vector.tensor_mul(ab[:, 0:CT], ps_bc[:, CT : 2 * CT], gam[:])
                tb = wp.tile([128, CT], F32, name=f"tb{b}", tag="tb", bufs=2)
                nc.vector.tensor_mul(tb[:], ps_bc[:, 0:CT], ab[:, 0:CT])
                nc.vector.tensor_sub(ab[:, CT : 2 * CT], bet[:], tb[:])

            def gn_apply(b, ct):
                # xn = a*x + b in the compute dtype, spread over THREE
                # engines (Pool, DVE, ACT-Identity) so all four chunks
                # drain in ~2 tile-times.
                s = st[b]
                if "xn" not in s:
                    s["xn"] = wp.tile([128, CT, HW], CDT, name=f"xn{b}", tag="xn", bufs=2)
                ab = s["ab"]
                eng = nc.gpsimd if ct in (0, 3) else nc.vector
                eng.tensor_scalar(
                    out=s["xn"][:, ct, :], in0=s["x"][:, ct, :],
                    scalar1=ab[:, ct : ct + 1], scalar2=ab[:, CT + ct : CT + ct + 1],
                    op0=Alu.mult, op1=Alu.add,
                )

            def mm_k(ps, lhs_fn, rhs_fn, nk):
                for k in range(nk):
                    nc.tensor.matmul(
                        ps[:], lhs_fn(k), rhs_fn(k),
                        start=(k == 0), stop=(k == nk - 1),
                        perf_mode=DR,
                    )

            def ksl(t, k, lo, hi):
                return t[:, KSTEP * k : KSTEP * (k + 1), lo:hi]

            def t_mm(b, oc, nh):
                # t[:, oc, nh-half] = (M1 xn)[oc-chunk, half]  (+ c1 if biased)
                s = st[b]
                if "t" not in s:
                    s["t"] = wp.tile([128, CT, HW], CDT, name=f"t{b}", tag="t", bufs=2)
                ps = pp.tile([128, 512], F32, name=f"pt{b}_{oc}_{nh}", tag="mm", bufs=7)
                mm_k(ps,
                     lambda k: ksl(m1, k, oc * 128, (oc + 1) * 128),
                     lambda k: ksl(s["xn"], k, nh * 512, (nh + 1) * 512), NK)
                dst = s["t"][:, oc, nh * 512 : (nh + 1) * 512]
                if need_bias:
                    nc.scalar.activation(dst, ps[:], Act.Identity, bias=c1[:, oc : oc + 1])
                else:
                    nc.vector.tensor_copy(dst, ps[:])

            def ut_mm(b, mb):
                # ut[:, mb, :] = (xn^T M2^T)[mb-block, :]
                s = st[b]
                if "ut" not in s:
                    s["ut"] = wp.tile([128, NBLK, C], CDT, name=f"ut{b}", tag="ut", bufs=2)
                ps = pp.tile([128, 512], F32, name=f"pu{b}_{mb}", tag="mm", bufs=7)
                mm_k(ps,
                     lambda k: ksl(s["xn"], k, mb * 128, (mb + 1) * 128),
                     lambda k: ksl(m2, k, 0, C), NK)
                if b == 0:
                    nc.scalar.copy(s["ut"][:, mb, :], ps[:])
                else:
                    # DVE evac for b1: keeps ACT free to stream the b1 exps
                    # back-to-back (z/o of the kernel tail gate on them)
                    nc.vector.tensor_copy(s["ut"][:, mb, :], ps[:])

            def rx_mm(b):
                # general-bias path: rx_t[p, mb] = sum_c wr[c] xn[c, m]; the
                # per-key exp bias is INVSQ*rx - SHIFT (+ bq.bk const).
                s = st[b]
                s["rxb"] = wp.tile([128, NBLK], F32, name=f"rxb{b}", tag="rxb", bufs=2)
                for mb in range(NBLK):
                    ps = pp.tile([128, 1], F32, name=f"prx{b}_{mb}", tag="gbc", bufs=1)
                    mm_k(ps,
                         lambda k: ksl(s["xn"], k, mb * 128, (mb + 1) * 128),
                         lambda k: ksl(wr, k, 0, 1), NK)
                    nc.vector.tensor_scalar(
                        out=s["rxb"][:, mb : mb + 1], in0=ps[:],
                        scalar1=INVSQ, scalar2=RXCONST[0] - SHIFT,
                        op0=Alu.mult, op1=Alu.add,
                    )

            def sc_mm(b, mb, nh):
                # scores^T tile [m-block, n-half] + exp -> E fp8
                s = st[b]
                if "E" not in s:
                    s["E"] = wp.tile([128, NBLK, HW], CDT, name=f"E{b}", tag="E", bufs=2)
                ps = pp.tile([128, 512], F32, name=f"psc{b}_{mb}_{nh}", tag="mm", bufs=7)
                mm_k(ps,
                     lambda k: ksl(s["t"], k, mb * 128, (mb + 1) * 128),
                     lambda k: ksl(s["xn"], k, nh * 512, (nh + 1) * 512), NK)
                bias = s["rxb"][:, mb : mb + 1] if need_bias else ebias[:, 0:1]
                nc.scalar.activation(
                    s["E"][:, mb, nh * 512 : (nh + 1) * 512], ps[:],
                    Act.Exp, bias=bias, scale=INVSQ,
                )

            def z_mm(b, nh):
                # Z[n] on ALL partitions at once: ones[128x128]-stationary
                # matmul accumulated over the 8 m-blocks; then 1/Z on DVE.
                s = st[b]
                if "zr" not in s:
                    s["zr"] = wp.tile([128, 2, 512], F32, name=f"zr{b}", tag="zr", bufs=2)
                ps = pp.tile([128, 512], F32, name=f"psz{b}_{nh}", tag="mm", bufs=7)
                mm_k(ps,
                     lambda k: ones_z[:] if FP8 else ones_z[:, 0, :],
                     lambda k: ksl(s["E"], k, nh * 512, (nh + 1) * 512), NM)
                nc.vector.reciprocal_approx_fast(s["zr"][:, nh, :], ps[:])

            def o_mm(b, ct, nh, fine=False):
                # out[ct-chunk, nh-half] = outU * zr (+ b_f) + skip, streamed
                # out. fine=True evacuates in 2x256 halves (shorter chain on
                # the kernel's very last tiles).
                s = st[b]
                out_r = out_d[b].rearrange("(ct p) n -> p ct n", p=128)
                ps = pp.tile([128, 512], F32, name=f"po{b}_{ct}_{nh}", tag="mm", bufs=7)
                mm_k(ps,
                     lambda k: ksl(s["ut"], k, ct * 128, (ct + 1) * 128),
                     lambda k: ksl(s["E"], k, nh * 512, (nh + 1) * 512), NM)
                tmp = wp.tile([128, 512], F32, name=f"tmp{b}_{ct}_{nh}", tag="tmp", bufs=6)
                nhalf = 2 if fine else 1
                for hh in range(nhalf):
                    w = 512 // nhalf
                    fsl = slice(hh * w, (hh + 1) * w)
                    sl = s["x"][:, ct, nh * 512 + hh * w : nh * 512 + (hh + 1) * w]
                    nc.vector.tensor_tensor(
                        tmp[:, fsl], ps[:, fsl], s["zr"][:, nh, fsl], op=Alu.mult)
                    if need_bias:
                        nc.vector.scalar_tensor_tensor(
                            out=tmp[:, fsl], in0=tmp[:, fsl],
                            scalar=b_f[:, ct : ct + 1], in1=sl,
                            op0=Alu.add, op1=Alu.add,
                        )
                    else:
                        # alternate Pool/DVE so consecutive skip-adds overlap
                        eng = nc.gpsimd if (ct + hh) % 2 == 0 else nc.vector
                        eng.tensor_tensor(tmp[:, fsl], tmp[:, fsl], sl, op=Alu.add)
                    nc.sync.dma_start(
                        out_r[:, ct, nh * 512 + hh * w : nh * 512 + (hh + 1) * w],
                        tmp[:, fsl])

            # ---- DMA emission order == HWDGE queue order: all of x0, then
            # m1 (needed ~when t(0) starts), then x1, then m2 (needed at
            # ut(0), which runs after sc(0,h0)).
            for b in range(BL):
                st[b]["x"] = wp.tile([128, CT, HW], BF16, name=f"x{b}", tag="x", bufs=2)
            x_r = [x_d[b].rearrange("(ct p) n -> p ct n", p=128) for b in range(BL)]
            for ct in range(CT):
                nc.sync.dma_start(st[0]["x"][:, ct, :], x_r[0][:, ct, :])
            m1 = cp.tile([128, CT, C], CDT, name="m1", tag="m1")
            nc.sync.dma_start(m1[:], m1_d.rearrange("(kc p) o -> p kc o", p=128))
            for ct in range(CT):
                nc.sync.dma_start(st[1]["x"][:, ct, :], x_r[1][:, ct, :])
            m2 = cp.tile([128, CT, C], CDT, name="m2", tag="m2")
            nc.sync.dma_start(m2[:], m2_d.rearrange("(kc p) o -> p kc o", p=128))

            # ---- emission order == scheduler priority ----
            # PE queue: warm-ups, gn(0) tiny mms, t(0), sc(0,h0), gn(1)
            # tiny mms (their stats are ready by then - no head-of-line
            # block), ut(0), sc(0,h1), t(1), then the attention tail.
            warmup(NWARM)
            for ct in range(CT):
                gn_stats(0, ct)
                warm_x(0, ct)
                warm_x(0, ct)
                warm_x(0, ct)
                warm_x(0, ct)
            gn_chain_a(0)
            warm_x(1, 0)
            warm_x(1, 1)
            gn_chain_b(0)
            for ct in range(CT):
                gn_apply(0, ct)
            for ct in range(CT):
                gn_stats(1, ct)
            warm_x(1, 2)
            warm_x(1, 3)
            if need_bias:
                rx_mm(0)
            for oc in range(CT):
                t_mm(0, oc, 0); t_mm(0, oc, 1)
            gn_chain_a(1)
            for mb in range(NBLK):
                sc_mm(0, mb, 0)
            gn_chain_b(1)
            for ct in range(CT):
                gn_apply(1, ct)
            for mb in range(NBLK):
                ut_mm(0, mb)
            for mb in range(NBLK):
                sc_mm(0, mb, 1)
            if need_bias:
                rx_mm(1)
            for oc in range(CT):
                t_mm(1, oc, 0); t_mm(1, oc, 1)
            z_mm(0, 0)
            for ct in range(CT):
                o_mm(0, ct, 0)
            z_mm(0, 1)
            for ct in range(CT):
                o_mm(0, ct, 1)
            for mb in range(NBLK):
                sc_mm(1, mb, 0)
            for mb in range(NBLK):
                ut_mm(1, mb)
            for mb in range(NBLK):
                sc_mm(1, mb, 1)
            z_mm(1, 0)
            for ct in range(CT):
                o_mm(1, ct, 0)
            z_mm(1, 1)
            for ct in range(CT):
                o_mm(1, ct, 1, fine=(ct >= 2))

    nc.compile()
    return nc


# constant exp-bias addend for the general-bias path (bq.bk term);
# set by _make_in_maps before the program is built
RXCONST = [0.0]


def _get_program(need_bias):
    key = (V2DT, SHIFT, need_bias, NEWTON, NWARM)
    if key not in _CACHE:
        _CACHE[key] = _build_program(need_bias)
    return _CACHE[key]


def _to_compute(a):
    """Convert host fp32 weights to the matmul compute format."""
    import ml_dtypes
    a = np.ascontiguousarray(a, dtype=np.float32)
    if V2DT == "fp8":
        return np.ascontiguousarray(a.astype(ml_dtypes.float8_e4m3))
    return np.ascontiguousarray(a.astype(ml_dtypes.bfloat16))


def _make_in_maps(x, gamma, beta, w_in, b_in, w_out, b_out):
    import ml_dtypes
    x = np.ascontiguousarray(
        np.asarray(x, dtype=np.float32).reshape(B, C, HW).astype(ml_dtypes.bfloat16)
    )
    w_in = np.asarray(w_in, dtype=np.float32)
    w_out = np.asarray(w_out, dtype=np.float32)
    b_in = np.asarray(b_in, dtype=np.float32)
    b_out = np.asarray(b_out, dtype=np.float32)
    wq, wk, wv = w_in[0:C], w_in[C : 2 * C], w_in[2 * C : 3 * C]
    bq, bk, bv = b_in[0:C], b_in[C : 2 * C], b_in[2 * C : 3 * C]
    m1 = wq.T @ wk                      # scores = xn^T m1 xn (+ bias terms)
    m2 = w_out @ wv                     # out = m2 xn attn^T + bf
    c1 = wq.T @ bk                      # q-side bias fold (per-channel)
    bf = w_out @ bv + b_out             # exact: softmax rows sum to 1
    wr = wk.T @ bq                      # k-side bias: varies along keys m
    need_bias = bool(np.any(c1) or np.any(bf) or np.any(wr) or np.any(bq))
    RXCONST[0] = float(INVSQ * np.dot(bq, bk))

    def cvec(v):
        return np.ascontiguousarray(v.reshape(CT, 128).T, dtype=np.float32)

    consts = {
        "m1t": _to_compute(m1.T),
        "m2t": _to_compute(m2.T),
        "gamma_t": cvec(np.asarray(gamma, dtype=np.float32)),
        "beta_t": cvec(np.asarray(beta, dtype=np.float32)),
        "ind_dn": (np.arange(128)[:, None] // GSIZE == np.arange(GSLOT)[None, :]).astype(np.float32),
        "ind_up": (np.arange(GSLOT)[:, None] == np.arange(128)[None, :] // GSIZE).astype(np.float32),
    }
    if need_bias:
        consts["c1_t"] = cvec(c1)
        consts["bf_t"] = cvec(bf)
        consts["wr_t"] = _to_compute(wr.reshape(CT, 128).T)
    return need_bias, [
        {"x": x[c * BL : (c + 1) * BL], **consts}
        for c in range(NCORES)
    ]


def run(inputs, trace=False):
    """Run on 8 cores; returns (output [B,C,H,W], BassKernelResults)."""
    from concourse.bass_utils import run_bass_kernel_spmd

    need_bias, in_maps = _make_in_maps(**inputs)
    nc = _get_program(need_bias)
    res = run_bass_kernel_spmd(nc, in_maps, core_ids=list(range(NCORES)), trace=trace)
    out = np.concatenate([res.results[i]["out"] for i in range(NCORES)], axis=0)
    return out.reshape(B, C, H, W).astype(np.float32), res


def kernel(**inputs) -> np.ndarray:
    out, _ = run(inputs)
    return out
